# revision 7
# baseline (speedup 1.0000x reference)
"""Distributed single-head attention kernel for one TRN2 chip (8 NeuronCores).

Reference computation (B=4, T=4096, D=256):
    qkv = x @ W.T + b            # W rows interleaved q,k,v (stride 3)
    q, k, v = deinterleave(qkv)
    scores = q @ k.T / sqrt(D);  mask keys where attention_mask == 0
    out = softmax(scores) @ v

Sharding: core c handles batch c//2, query-half c%2 (2048 queries).
Each core computes K/V over the full key sequence of its batch — no
collectives needed.  Host-side prep: de-interleave W, transpose x
(so the contraction dim lands on SBUF partitions), cast to bf16.

Device algorithm (per core), all matmuls bf16 with fp32 PSUM:
  KT[d, s] = Wk @ xT   (+bk per-partition, folded into the PSUM->SBUF copy)
  QT[d, t] = Wq @ xqT  (+bq likewise)
  V[s, dv+1] = (x @ Wv.T + bv | 1) * mask[s]   (ones col -> denominator;
               mask folded into V rows => exact masking, no -inf needed)
  per key block sb (128 keys) x query block qb (512 queries):
    scT[s,t] = KT.T-slices @ QT   (PSUM, 2 d-chunk accumulation)
    E = exp(scT * 1/sqrt(D))      (ScalarE, PSUM->SBUF, bf16 out;
                                   no max-subtraction: scores bounded)
    AV[t, dv+1] += E-slices.T @ V (PSUM accumulate over all key blocks)
  out[t, :] = AV[t, :256] * (1 / AV[t, 256])

A dummy-matmul warmup burst at t=0 brings the PE HAM clock gate to
8/8 (2.4 GHz) before the real work starts.
"""

import numpy as np
import ml_dtypes

import concourse.bass as bass
import concourse.mybir as mybir
import concourse.tile as tile
from concourse import bacc
from concourse.bass_utils import run_bass_kernel_spmd

B, T, D = 4, 4096, 256
TQ = T // 2              # queries per core
N_CORES = 8
SCALE = 1.0 / float(D) ** 0.5
P = 128
F32 = mybir.dt.float32

# attention compute dtype for matmul inputs
DT = mybir.dt.bfloat16
NPDT = ml_dtypes.bfloat16

# Compact keys by the attention mask on the host (shrinks the key range;
# exact — masked keys contribute exactly zero either way).
COMPACT = True

_nc_cache: dict = {}


def _build(tk: int):
    """Build the per-core graph for a key-sequence length of tk."""
    nc = bacc.Bacc("TRN2", target_bir_lowering=False, debug=False,
                   num_devices=1)
    nsb = tk // P            # key blocks
    nqb = TQ // 512          # query blocks of 512

    xkT_e = nc.dram_tensor("xkT", [D, tk], DT, kind="ExternalInput")
    xqT_e = nc.dram_tensor("xqT", [D, TQ], DT, kind="ExternalInput")
    w_e = nc.dram_tensor("wcat", [D, 769], DT, kind="ExternalInput")
    bias_e = nc.dram_tensor("biasones", [1, 769 + 1024], DT, kind="ExternalInput")
    bcol_e = nc.dram_tensor("biascols", [P, 4], F32, kind="ExternalInput")
    mask_e = nc.dram_tensor("maskf", [P, nsb], F32, kind="ExternalInput")
    out_e = nc.dram_tensor("out", [TQ, D], F32, kind="ExternalOutput")

    Exp = mybir.ActivationFunctionType.Exp

    with tile.TileContext(nc) as tc:
        with (
            tc.tile_pool(name="const", bufs=1) as cpool,
            tc.tile_pool(name="big", bufs=1) as bigpool,
            tc.tile_pool(name="exp", bufs=4) as epool,
            tc.tile_pool(name="fin", bufs=4) as fpool,
            tc.tile_pool(name="ps_sc", bufs=4, space="PSUM") as scpool,
            tc.tile_pool(name="ps_av", bufs=4, space="PSUM") as avpool,
        ):
            # ---- PE warmup: dummy matmuls to flip the HAM clock to 8/8 ----
            wu = cpool.tile([P, 512], DT, tag="wu")
            nc.gpsimd.memset(wu[:], 0.125)
            wups = scpool.tile([P, 512], F32, tag="sc", name="wups")
            NWU = 12
            for i in range(NWU):
                nc.tensor.matmul(wups[:], lhsT=wu[:, 0:P], rhs=wu[:],
                                 start=(i == 0), stop=(i == NWU - 1))

            # ---- constants ----
            w_sb = cpool.tile([P, 2 * 769], DT, tag="w")
            nc.sync.dma_start(out=w_sb[:, 0:769], in_=w_e[0:P, :])
            nc.sync.dma_start(out=w_sb[:, 769:1538], in_=w_e[P:D, :])
            bias_sb = cpool.tile([1, 769 + 1024], DT, tag="bias")
            nc.sync.dma_start(out=bias_sb[:], in_=bias_e[:])
            bcol_sb = cpool.tile([P, 4], F32, tag="bcol")
            nc.sync.dma_start(out=bcol_sb[:], in_=bcol_e[:])
            mask_sb = cpool.tile([P, nsb], F32, tag="mask")
            nc.sync.dma_start(out=mask_sb[:], in_=mask_e[:])
            ones = bias_sb[0:1, 769:1793]

            def wslice(din: int, off: int, n: int):
                return w_sb[:, din * 769 + off: din * 769 + off + n]

            # ---- load pre-transposed activations ----
            xkT = []
            for d in range(2):
                t_ = bigpool.tile([P, tk], DT, tag=f"xkT{d}", name=f"xkT{d}")
                h = tk // 256 * 128
                nc.sync.dma_start(out=t_[:, 0:h],
                                  in_=xkT_e[d * P:(d + 1) * P, 0:h])
                nc.sync.dma_start(out=t_[:, h:tk],
                                  in_=xkT_e[d * P:(d + 1) * P, h:tk])
                xkT.append(t_)

            # ---- K/Q projections (bias added in the PSUM->SBUF copy) ----
            KT = [bigpool.tile([P, tk], DT, tag=f"KT{d}", name=f"KT{d}")
                  for d in range(2)]
            QT = [bigpool.tile([P, TQ], DT, tag=f"QT{d}", name=f"QT{d}")
                  for d in range(2)]
            kchunks = [(c * 512, min(512, tk - c * 512))
                       for c in range((tk + 511) // 512)]
            for dout in range(2):
                koff = 256 + dout * P
                for c0, cn in kchunks:
                    ps = scpool.tile([P, 512], F32, tag="sc", name="kps")
                    sl = slice(c0, c0 + cn)
                    nc.tensor.matmul(ps[:, 0:cn], lhsT=wslice(0, koff, P),
                                     rhs=xkT[0][:, sl], start=True, stop=False)
                    nc.tensor.matmul(ps[:, 0:cn], lhsT=wslice(1, koff, P),
                                     rhs=xkT[1][:, sl], start=False, stop=True)
                    nc.scalar.add(KT[dout][:, sl], ps[:, 0:cn],
                                  bcol_sb[:, 2 + dout:3 + dout])

            # ---- V projection: V[s, 257] = (x@Wv.T + bv | 1) * mask ----
            V = bigpool.tile([P, nsb * 257], DT, tag="V")
            for sb in range(nsb):
                ps = avpool.tile([P, 257], F32, tag="av", name="vps")
                ssl = slice(sb * P, (sb + 1) * P)
                nc.tensor.matmul(ps[:], lhsT=xkT[0][:, ssl],
                                 rhs=wslice(0, 512, 257), start=True, stop=False)
                nc.tensor.matmul(ps[:], lhsT=xkT[1][:, ssl],
                                 rhs=wslice(1, 512, 257), start=False, stop=False)
                nc.tensor.matmul(ps[:], lhsT=ones[0:1, 0:P],
                                 rhs=bias_sb[0:1, 512:769], start=False, stop=True)
                nc.vector.tensor_scalar_mul(V[:, sb * 257:(sb + 1) * 257], ps[:],
                                            mask_sb[:, sb:sb + 1])

            # ---- Q path (after K/V so the K DMAs/projections go first) ----
            xqT = []
            for d in range(2):
                t_ = bigpool.tile([P, TQ], DT, tag=f"xqT{d}", name=f"xqT{d}")
                for h in range(2):
                    hs = slice(h * TQ // 2, (h + 1) * TQ // 2)
                    nc.sync.dma_start(out=t_[:, hs],
                                      in_=xqT_e[d * P:(d + 1) * P, hs])
                xqT.append(t_)
            for dout in range(2):
                qoff = dout * P
                for c in range(TQ // 512):
                    ps = scpool.tile([P, 512], F32, tag="sc", name="qps")
                    sl = slice(c * 512, (c + 1) * 512)
                    nc.tensor.matmul(ps[:], lhsT=wslice(0, qoff, P),
                                     rhs=xqT[0][:, sl], start=True, stop=False)
                    nc.tensor.matmul(ps[:], lhsT=wslice(1, qoff, P),
                                     rhs=xqT[1][:, sl], start=False, stop=True)
                    nc.scalar.add(QT[dout][:, sl], ps[:],
                                  bcol_sb[:, dout:dout + 1])

            # ---- attention ----
            for qb in range(nqb):
                qsl = slice(qb * 512, (qb + 1) * 512)
                av = [avpool.tile([P, 257], F32, tag="av", name=f"av{qb}_{i}")
                      for i in range(4)]

                def emit_av(et, sb):
                    for tt in range(4):
                        nc.tensor.matmul(
                            av[tt][:],
                            lhsT=et[:, tt * P:(tt + 1) * P],
                            rhs=V[:, sb * 257:(sb + 1) * 257],
                            start=(sb == 0), stop=(sb == nsb - 1))

                pending = None
                for sb in range(nsb):
                    sc = scpool.tile([P, 512], F32, tag="sc", name="sc")
                    ssl = slice(sb * P, (sb + 1) * P)
                    nc.tensor.matmul(sc[:], lhsT=KT[0][:, ssl],
                                     rhs=QT[0][:, qsl], start=True, stop=False)
                    nc.tensor.matmul(sc[:], lhsT=KT[1][:, ssl],
                                     rhs=QT[1][:, qsl], start=False, stop=True)
                    et = epool.tile([P, 512], DT, tag="exp", name="et")
                    nc.scalar.activation(et[:], sc[:], Exp, scale=SCALE)
                    if pending is not None:
                        emit_av(*pending)
                    pending = (et, sb)
                emit_av(*pending)

                # ---- normalize + store ----
                for tt in range(4):
                    rec = fpool.tile([P, 1], F32, tag="rec", name="rec")
                    nc.vector.reciprocal(rec[:], av[tt][:, 256:257])
                    ot = fpool.tile([P, D], F32, tag="ot", name="ot")
                    nc.vector.tensor_scalar_mul(ot[:], av[tt][:, 0:256], rec[:])
                    r0 = (qb * 4 + tt) * P
                    nc.sync.dma_start(out=out_e[r0:r0 + P, :], in_=ot[:])
    nc.finalize()
    return nc


def _get_nc(tk: int):
    if tk not in _nc_cache:
        _nc_cache[tk] = _build(tk)
    return _nc_cache[tk]


def _prep_in_maps(x, W, b, attention_mask):
    """Host-side layout prep + sharding. Returns (in_maps, tk)."""
    # de-interleave: W row 3d+n is (q,k,v)[n] output-dim d
    Wq, Wk, Wv = W[0::3], W[1::3], W[2::3]          # each [D, D]
    bq, bk, bv = b[0::3], b[1::3], b[2::3]
    wcat = np.concatenate(
        [Wq.T, Wk.T, Wv.T, np.zeros((D, 1), np.float32)], axis=1)
    biasones = np.concatenate(
        [np.zeros(512, np.float32), bv, [1.0],
         np.ones(1024, np.float32)])[None, :]
    wcat = np.ascontiguousarray(wcat).astype(NPDT)
    biasones = np.ascontiguousarray(biasones).astype(NPDT)
    biascols = np.ascontiguousarray(
        np.stack([bq[:P], bq[P:], bk[:P], bk[P:]], axis=1).astype(np.float32))

    if COMPACT:
        keep = [np.nonzero(attention_mask[bi] != 0)[0] for bi in range(B)]
        max_keep = max(len(k) for k in keep)
        tk = max(256, ((max_keep + 127) // 128) * 128)
    else:
        keep = None
        tk = T

    xkT_b, mask_b = [], []
    for bi in range(B):
        xb = x[bi]                                   # [T, D] f32
        if COMPACT:
            k = keep[bi]
            xs = np.zeros((tk, D), np.float32)
            xs[:len(k)] = xb[k]
            mf = np.zeros(tk, np.float32)
            mf[:len(k)] = 1.0
        else:
            xs = xb
            mf = attention_mask[bi].astype(np.float32)
        xkT_b.append(np.ascontiguousarray(xs.T.astype(NPDT)))
        mask_b.append(np.ascontiguousarray(
            mf.reshape(tk // P, P).T.astype(np.float32)))  # [P, nsb]

    in_maps = []
    for c in range(N_CORES):
        bi, half = c // 2, c % 2
        xqT = np.ascontiguousarray(
            x[bi][half * TQ:(half + 1) * TQ].T.astype(NPDT))
        in_maps.append({
            "xkT": xkT_b[bi],
            "xqT": xqT,
            "wcat": wcat,
            "biasones": biasones,
            "biascols": biascols,
            "maskf": mask_b[bi],
        })
    return in_maps, tk


def _gather(results) -> np.ndarray:
    out = np.empty((B, T, D), np.float32)
    for c in range(N_CORES):
        bi, half = c // 2, c % 2
        out[bi, half * TQ:(half + 1) * TQ] = results[c]["out"]
    return out


def kernel(x, W, b, attention_mask) -> np.ndarray:
    x = np.asarray(x, np.float32)
    W = np.asarray(W, np.float32)
    b = np.asarray(b, np.float32)
    attention_mask = np.asarray(attention_mask)
    in_maps, tk = _prep_in_maps(x, W, b, attention_mask)
    nc = _get_nc(tk)
    res = run_bass_kernel_spmd(nc, in_maps, list(range(N_CORES)))
    return _gather(res.results)


# revision 8
# speedup vs baseline: 1.0607x; 1.0607x over previous
"""Distributed single-head attention kernel for one TRN2 chip (8 NeuronCores).

Reference computation (B=4, T=4096, D=256):
    qkv = x @ W.T + b            # W rows interleaved q,k,v (stride 3)
    q, k, v = deinterleave(qkv)
    scores = q @ k.T / sqrt(D);  mask keys where attention_mask == 0
    out = softmax(scores) @ v

Sharding: core c handles batch c//2, query-half c%2 (2048 queries).
Each core computes K/V over the full key sequence of its batch — no
collectives needed.  Host-side prep: de-interleave W, transpose x
(so the contraction dim lands on SBUF partitions), cast to bf16.

Device algorithm (per core), all matmuls bf16 with fp32 PSUM:
  KT[d, s] = Wk @ xT   (+bk per-partition, folded into the PSUM->SBUF copy)
  QT[d, t] = Wq @ xqT  (+bq likewise)
  V[s, dv+1] = (x @ Wv.T + bv | 1) * mask[s]   (ones col -> denominator;
               mask folded into V rows => exact masking, no -inf needed)
  per key block sb (128 keys) x query block qb (512 queries):
    scT[s,t] = KT.T-slices @ QT   (PSUM, 2 d-chunk accumulation)
    E = exp(scT * 1/sqrt(D))      (ScalarE, PSUM->SBUF, bf16 out;
                                   no max-subtraction: scores bounded)
    AV[t, dv+1] += E-slices.T @ V (PSUM accumulate over all key blocks)
  out[t, :] = AV[t, :256] * (1 / AV[t, 256])

A dummy-matmul warmup burst at t=0 brings the PE HAM clock gate to
8/8 (2.4 GHz) before the real work starts.
"""

import numpy as np
import ml_dtypes

import concourse.bass as bass
import concourse.mybir as mybir
import concourse.tile as tile
from concourse import bacc
from concourse.bass_utils import run_bass_kernel_spmd

B, T, D = 4, 4096, 256
TQ = T // 2              # queries per core
N_CORES = 8
SCALE = 1.0 / float(D) ** 0.5
P = 128
F32 = mybir.dt.float32

# attention compute dtype for matmul inputs
DT = mybir.dt.bfloat16
NPDT = ml_dtypes.bfloat16

# Compact keys by the attention mask on the host (shrinks the key range;
# exact — masked keys contribute exactly zero either way).
COMPACT = True

_nc_cache: dict = {}


def _build(tk: int):
    """Build the per-core graph for a key-sequence length of tk."""
    nc = bacc.Bacc("TRN2", target_bir_lowering=False, debug=False,
                   num_devices=1)
    nsb = tk // P            # key blocks
    nqb = TQ // 512          # query blocks of 512

    xkT_e = nc.dram_tensor("xkT", [D, tk], DT, kind="ExternalInput")
    xqT_e = nc.dram_tensor("xqT", [D, TQ], DT, kind="ExternalInput")
    w_e = nc.dram_tensor("wcat", [D, 769], DT, kind="ExternalInput")
    bias_e = nc.dram_tensor("biasones", [1, 769 + 1024], DT, kind="ExternalInput")
    bcol_e = nc.dram_tensor("biascols", [P, 4], F32, kind="ExternalInput")
    mask_e = nc.dram_tensor("maskf", [P, nsb], F32, kind="ExternalInput")
    out_e = nc.dram_tensor("out", [TQ, D], F32, kind="ExternalOutput")

    Exp = mybir.ActivationFunctionType.Exp

    with tile.TileContext(nc) as tc:
        with (
            tc.tile_pool(name="const", bufs=1) as cpool,
            tc.tile_pool(name="big", bufs=1) as bigpool,
            tc.tile_pool(name="exp", bufs=4) as epool,
            tc.tile_pool(name="fin", bufs=4) as fpool,
            tc.tile_pool(name="ps_sc", bufs=4, space="PSUM") as scpool,
            tc.tile_pool(name="ps_av", bufs=4, space="PSUM") as avpool,
        ):
            # ---- PE warmup: dummy matmuls to flip the HAM clock to 8/8 ----
            wu = cpool.tile([P, 512], DT, tag="wu")
            nc.gpsimd.memset(wu[:], 0.125)
            wups = scpool.tile([P, 512], F32, tag="sc", name="wups")
            NWU = 28
            for i in range(NWU):
                nc.tensor.matmul(wups[:], lhsT=wu[:, 0:P], rhs=wu[:],
                                 start=(i == 0), stop=(i == NWU - 1))

            # ---- constants ----
            w_sb = cpool.tile([P, 2 * 769], DT, tag="w")
            nc.sync.dma_start(out=w_sb[:, 0:769], in_=w_e[0:P, :])
            nc.sync.dma_start(out=w_sb[:, 769:1538], in_=w_e[P:D, :])
            bias_sb = cpool.tile([1, 769 + 1024], DT, tag="bias")
            nc.sync.dma_start(out=bias_sb[:], in_=bias_e[:])
            bcol_sb = cpool.tile([P, 4], F32, tag="bcol")
            nc.sync.dma_start(out=bcol_sb[:], in_=bcol_e[:])
            mask_sb = cpool.tile([P, nsb], F32, tag="mask")
            nc.sync.dma_start(out=mask_sb[:], in_=mask_e[:])
            ones = bias_sb[0:1, 769:1793]

            def wslice(din: int, off: int, n: int):
                return w_sb[:, din * 769 + off: din * 769 + off + n]

            # ---- load pre-transposed activations ----
            xkT = []
            for d in range(2):
                t_ = bigpool.tile([P, tk], DT, tag=f"xkT{d}", name=f"xkT{d}")
                nc.sync.dma_start(out=t_[:], in_=xkT_e[d * P:(d + 1) * P, :])
                xkT.append(t_)

            # ---- K/Q projections (bias added in the PSUM->SBUF copy) ----
            KT = [bigpool.tile([P, tk], DT, tag=f"KT{d}", name=f"KT{d}")
                  for d in range(2)]
            QT = [bigpool.tile([P, TQ], DT, tag=f"QT{d}", name=f"QT{d}")
                  for d in range(2)]
            kchunks = [(c * 512, min(512, tk - c * 512))
                       for c in range((tk + 511) // 512)]
            for dout in range(2):
                koff = 256 + dout * P
                for c0, cn in kchunks:
                    ps = scpool.tile([P, 512], F32, tag="sc", name="kps")
                    sl = slice(c0, c0 + cn)
                    nc.tensor.matmul(ps[:, 0:cn], lhsT=wslice(0, koff, P),
                                     rhs=xkT[0][:, sl], start=True, stop=False)
                    nc.tensor.matmul(ps[:, 0:cn], lhsT=wslice(1, koff, P),
                                     rhs=xkT[1][:, sl], start=False, stop=True)
                    nc.scalar.add(KT[dout][:, sl], ps[:, 0:cn],
                                  bcol_sb[:, 2 + dout:3 + dout])

            # ---- V projection: V[s, 257] = (x@Wv.T + bv | 1) * mask ----
            V = bigpool.tile([P, nsb * 257], DT, tag="V")
            for sb in range(nsb):
                ps = avpool.tile([P, 257], F32, tag="av", name="vps")
                ssl = slice(sb * P, (sb + 1) * P)
                nc.tensor.matmul(ps[:], lhsT=xkT[0][:, ssl],
                                 rhs=wslice(0, 512, 257), start=True, stop=False)
                nc.tensor.matmul(ps[:], lhsT=xkT[1][:, ssl],
                                 rhs=wslice(1, 512, 257), start=False, stop=False)
                nc.tensor.matmul(ps[:], lhsT=ones[0:1, 0:P],
                                 rhs=bias_sb[0:1, 512:769], start=False, stop=True)
                nc.vector.tensor_scalar_mul(V[:, sb * 257:(sb + 1) * 257], ps[:],
                                            mask_sb[:, sb:sb + 1])

            # ---- Q path (after K/V so the K DMAs/projections go first) ----
            xqT = []
            for d in range(2):
                t_ = bigpool.tile([P, TQ], DT, tag=f"xqT{d}", name=f"xqT{d}")
                nc.sync.dma_start(out=t_[:], in_=xqT_e[d * P:(d + 1) * P, :])
                xqT.append(t_)
            for dout in range(2):
                qoff = dout * P
                for c in range(TQ // 512):
                    ps = scpool.tile([P, 512], F32, tag="sc", name="qps")
                    sl = slice(c * 512, (c + 1) * 512)
                    nc.tensor.matmul(ps[:], lhsT=wslice(0, qoff, P),
                                     rhs=xqT[0][:, sl], start=True, stop=False)
                    nc.tensor.matmul(ps[:], lhsT=wslice(1, qoff, P),
                                     rhs=xqT[1][:, sl], start=False, stop=True)
                    nc.scalar.add(QT[dout][:, sl], ps[:],
                                  bcol_sb[:, dout:dout + 1])

            # ---- attention ----
            for qb in range(nqb):
                qsl = slice(qb * 512, (qb + 1) * 512)
                av = [avpool.tile([P, 257], F32, tag="av", name=f"av{qb}_{i}")
                      for i in range(4)]

                def emit_av(et, sb):
                    for tt in range(4):
                        nc.tensor.matmul(
                            av[tt][:],
                            lhsT=et[:, tt * P:(tt + 1) * P],
                            rhs=V[:, sb * 257:(sb + 1) * 257],
                            start=(sb == 0), stop=(sb == nsb - 1))

                pending = None
                for sb in range(nsb):
                    sc = scpool.tile([P, 512], F32, tag="sc", name="sc")
                    ssl = slice(sb * P, (sb + 1) * P)
                    nc.tensor.matmul(sc[:], lhsT=KT[0][:, ssl],
                                     rhs=QT[0][:, qsl], start=True, stop=False)
                    nc.tensor.matmul(sc[:], lhsT=KT[1][:, ssl],
                                     rhs=QT[1][:, qsl], start=False, stop=True)
                    et = epool.tile([P, 512], DT, tag="exp", name="et")
                    nc.scalar.activation(et[:], sc[:], Exp, scale=SCALE)
                    if pending is not None:
                        emit_av(*pending)
                    pending = (et, sb)
                emit_av(*pending)

                # ---- normalize + store ----
                for tt in range(4):
                    rec = fpool.tile([P, 1], F32, tag="rec", name="rec")
                    nc.vector.reciprocal(rec[:], av[tt][:, 256:257])
                    ot = fpool.tile([P, D], F32, tag="ot", name="ot")
                    nc.vector.tensor_scalar_mul(ot[:], av[tt][:, 0:256], rec[:])
                    r0 = (qb * 4 + tt) * P
                    nc.sync.dma_start(out=out_e[r0:r0 + P, :], in_=ot[:])
    nc.finalize()
    return nc


def _get_nc(tk: int):
    if tk not in _nc_cache:
        _nc_cache[tk] = _build(tk)
    return _nc_cache[tk]


def _prep_in_maps(x, W, b, attention_mask):
    """Host-side layout prep + sharding. Returns (in_maps, tk)."""
    # de-interleave: W row 3d+n is (q,k,v)[n] output-dim d
    Wq, Wk, Wv = W[0::3], W[1::3], W[2::3]          # each [D, D]
    bq, bk, bv = b[0::3], b[1::3], b[2::3]
    wcat = np.concatenate(
        [Wq.T, Wk.T, Wv.T, np.zeros((D, 1), np.float32)], axis=1)
    biasones = np.concatenate(
        [np.zeros(512, np.float32), bv, [1.0],
         np.ones(1024, np.float32)])[None, :]
    wcat = np.ascontiguousarray(wcat).astype(NPDT)
    biasones = np.ascontiguousarray(biasones).astype(NPDT)
    biascols = np.ascontiguousarray(
        np.stack([bq[:P], bq[P:], bk[:P], bk[P:]], axis=1).astype(np.float32))

    if COMPACT:
        keep = [np.nonzero(attention_mask[bi] != 0)[0] for bi in range(B)]
        max_keep = max(len(k) for k in keep)
        tk = max(256, ((max_keep + 127) // 128) * 128)
    else:
        keep = None
        tk = T

    xkT_b, mask_b = [], []
    for bi in range(B):
        xb = x[bi]                                   # [T, D] f32
        if COMPACT:
            k = keep[bi]
            xs = np.zeros((tk, D), np.float32)
            xs[:len(k)] = xb[k]
            mf = np.zeros(tk, np.float32)
            mf[:len(k)] = 1.0
        else:
            xs = xb
            mf = attention_mask[bi].astype(np.float32)
        xkT_b.append(np.ascontiguousarray(xs.T.astype(NPDT)))
        mask_b.append(np.ascontiguousarray(
            mf.reshape(tk // P, P).T.astype(np.float32)))  # [P, nsb]

    in_maps = []
    for c in range(N_CORES):
        bi, half = c // 2, c % 2
        xqT = np.ascontiguousarray(
            x[bi][half * TQ:(half + 1) * TQ].T.astype(NPDT))
        in_maps.append({
            "xkT": xkT_b[bi],
            "xqT": xqT,
            "wcat": wcat,
            "biasones": biasones,
            "biascols": biascols,
            "maskf": mask_b[bi],
        })
    return in_maps, tk


def _gather(results) -> np.ndarray:
    out = np.empty((B, T, D), np.float32)
    for c in range(N_CORES):
        bi, half = c // 2, c % 2
        out[bi, half * TQ:(half + 1) * TQ] = results[c]["out"]
    return out


def kernel(x, W, b, attention_mask) -> np.ndarray:
    x = np.asarray(x, np.float32)
    W = np.asarray(W, np.float32)
    b = np.asarray(b, np.float32)
    attention_mask = np.asarray(attention_mask)
    in_maps, tk = _prep_in_maps(x, W, b, attention_mask)
    nc = _get_nc(tk)
    res = run_bass_kernel_spmd(nc, in_maps, list(range(N_CORES)))
    return _gather(res.results)


# revision 9
# speedup vs baseline: 1.0890x; 1.0267x over previous
"""Distributed single-head attention kernel for one TRN2 chip (8 NeuronCores).

Reference computation (B=4, T=4096, D=256):
    qkv = x @ W.T + b            # W rows interleaved q,k,v (stride 3)
    q, k, v = deinterleave(qkv)
    scores = q @ k.T / sqrt(D);  mask keys where attention_mask == 0
    out = softmax(scores) @ v

Sharding: core c handles batch c//2, query-half c%2 (2048 queries).
Each core computes K/V over the full key sequence of its batch — no
collectives needed.  Host-side prep: de-interleave W, transpose x
(so the contraction dim lands on SBUF partitions), cast to bf16.

Device algorithm (per core), all matmuls bf16 with fp32 PSUM:
  KT[d, s] = Wk @ xT   (+bk per-partition, folded into the PSUM->SBUF copy)
  QT[d, t] = Wq @ xqT  (+bq likewise)
  V[s, dv+1] = (x @ Wv.T + bv | 1) * mask[s]   (ones col -> denominator;
               mask folded into V rows => exact masking, no -inf needed)
  per key block sb (128 keys) x query block qb (512 queries):
    scT[s,t] = KT.T-slices @ QT   (PSUM, 2 d-chunk accumulation)
    E = exp(scT * 1/sqrt(D))      (ScalarE, PSUM->SBUF, bf16 out;
                                   no max-subtraction: scores bounded)
    AV[t, dv+1] += E-slices.T @ V (PSUM accumulate over all key blocks)
  out[t, :] = AV[t, :256] * (1 / AV[t, 256])

A dummy-matmul warmup burst at t=0 brings the PE HAM clock gate to
8/8 (2.4 GHz) before the real work starts.
"""

import numpy as np
import ml_dtypes

import concourse.bass as bass
import concourse.mybir as mybir
import concourse.tile as tile
from concourse import bacc
from concourse.bass_utils import run_bass_kernel_spmd

B, T, D = 4, 4096, 256
TQ = T // 2              # queries per core
N_CORES = 8
SCALE = 1.0 / float(D) ** 0.5
P = 128
F32 = mybir.dt.float32

# attention compute dtype for matmul inputs
DT = mybir.dt.bfloat16
NPDT = ml_dtypes.bfloat16

# Compact keys by the attention mask on the host (shrinks the key range;
# exact — masked keys contribute exactly zero either way).
COMPACT = True

_nc_cache: dict = {}


def _build(tk: int):
    """Build the per-core graph for a key-sequence length of tk."""
    nc = bacc.Bacc("TRN2", target_bir_lowering=False, debug=False,
                   num_devices=1)
    nsb = tk // P            # key blocks
    nqb = TQ // 512          # query blocks of 512

    xkT_e = nc.dram_tensor("xkT", [D, tk], DT, kind="ExternalInput")
    xqT_e = nc.dram_tensor("xqT", [D, TQ], DT, kind="ExternalInput")
    w_e = nc.dram_tensor("wcat", [D, 769], DT, kind="ExternalInput")
    bias_e = nc.dram_tensor("biasones", [1, 769 + 1024], DT, kind="ExternalInput")
    bcol_e = nc.dram_tensor("biascols", [P, 4], F32, kind="ExternalInput")
    mask_e = nc.dram_tensor("maskf", [P, nsb], F32, kind="ExternalInput")
    out_e = nc.dram_tensor("out", [TQ, D], F32, kind="ExternalOutput")

    Exp = mybir.ActivationFunctionType.Exp

    with tile.TileContext(nc) as tc:
        with (
            tc.tile_pool(name="const", bufs=1) as cpool,
            tc.tile_pool(name="big", bufs=1) as bigpool,
            tc.tile_pool(name="exp", bufs=4) as epool,
            tc.tile_pool(name="fin", bufs=4) as fpool,
            tc.tile_pool(name="ps_sc", bufs=4, space="PSUM") as scpool,
            tc.tile_pool(name="ps_av", bufs=4, space="PSUM") as avpool,
        ):
            # ---- PE warmup: dummy matmuls to flip the HAM clock to 8/8 ----
            wu = cpool.tile([P, 512], DT, tag="wu")
            nc.gpsimd.memset(wu[:], 0.125)
            wups = scpool.tile([P, 512], F32, tag="sc", name="wups")
            NWU = 20
            for i in range(NWU):
                nc.tensor.matmul(wups[:], lhsT=wu[:, 0:P], rhs=wu[:],
                                 start=(i == 0), stop=(i == NWU - 1))

            # ---- load pre-transposed activations (issued first) ----
            xkT = []
            for d in range(2):
                t_ = bigpool.tile([P, tk], DT, tag=f"xkT{d}", name=f"xkT{d}")
                nc.sync.dma_start(out=t_[:], in_=xkT_e[d * P:(d + 1) * P, :])
                xkT.append(t_)

            # ---- constants ----
            w_sb = cpool.tile([P, 2 * 769], DT, tag="w")
            nc.sync.dma_start(out=w_sb[:, 0:769], in_=w_e[0:P, :])
            nc.sync.dma_start(out=w_sb[:, 769:1538], in_=w_e[P:D, :])
            bias_sb = cpool.tile([1, 769 + 1024], DT, tag="bias")
            nc.sync.dma_start(out=bias_sb[:], in_=bias_e[:])
            bcol_sb = cpool.tile([P, 4], F32, tag="bcol")
            nc.sync.dma_start(out=bcol_sb[:], in_=bcol_e[:])
            mask_sb = cpool.tile([P, nsb], F32, tag="mask")
            nc.sync.dma_start(out=mask_sb[:], in_=mask_e[:])
            ones = bias_sb[0:1, 769:1793]

            def wslice(din: int, off: int, n: int):
                return w_sb[:, din * 769 + off: din * 769 + off + n]

            # ---- K/Q projections (bias added in the PSUM->SBUF copy) ----
            KT = [bigpool.tile([P, tk], DT, tag=f"KT{d}", name=f"KT{d}")
                  for d in range(2)]
            QT = [bigpool.tile([P, TQ], DT, tag=f"QT{d}", name=f"QT{d}")
                  for d in range(2)]
            kchunks = [(c * 512, min(512, tk - c * 512))
                       for c in range((tk + 511) // 512)]
            for dout in range(2):
                koff = 256 + dout * P
                for c0, cn in kchunks:
                    ps = scpool.tile([P, 512], F32, tag="sc", name="kps")
                    sl = slice(c0, c0 + cn)
                    nc.tensor.matmul(ps[:, 0:cn], lhsT=wslice(0, koff, P),
                                     rhs=xkT[0][:, sl], start=True, stop=False)
                    nc.tensor.matmul(ps[:, 0:cn], lhsT=wslice(1, koff, P),
                                     rhs=xkT[1][:, sl], start=False, stop=True)
                    if (c0 // 512) % 2 == 0:
                        nc.scalar.add(KT[dout][:, sl], ps[:, 0:cn],
                                      bcol_sb[:, 2 + dout:3 + dout])
                    else:
                        nc.vector.tensor_scalar_add(
                            KT[dout][:, sl], ps[:, 0:cn],
                            bcol_sb[:, 2 + dout:3 + dout])

            # ---- V projection: V[s, 257] = (x@Wv.T + bv | 1) * mask ----
            V = bigpool.tile([P, nsb * 257], DT, tag="V")
            for sb in range(nsb):
                ps = avpool.tile([P, 257], F32, tag="av", name="vps")
                ssl = slice(sb * P, (sb + 1) * P)
                nc.tensor.matmul(ps[:], lhsT=xkT[0][:, ssl],
                                 rhs=wslice(0, 512, 257), start=True, stop=False)
                nc.tensor.matmul(ps[:], lhsT=xkT[1][:, ssl],
                                 rhs=wslice(1, 512, 257), start=False, stop=False)
                nc.tensor.matmul(ps[:], lhsT=ones[0:1, 0:P],
                                 rhs=bias_sb[0:1, 512:769], start=False, stop=True)
                nc.vector.tensor_scalar_mul(V[:, sb * 257:(sb + 1) * 257], ps[:],
                                            mask_sb[:, sb:sb + 1])

            # ---- Q path (after K/V so the K DMAs/projections go first) ----
            xqT = []
            for d in range(2):
                t_ = bigpool.tile([P, TQ], DT, tag=f"xqT{d}", name=f"xqT{d}")
                nc.sync.dma_start(out=t_[:], in_=xqT_e[d * P:(d + 1) * P, :])
                xqT.append(t_)
            for dout in range(2):
                qoff = dout * P
                for c in range(TQ // 512):
                    ps = scpool.tile([P, 512], F32, tag="sc", name="qps")
                    sl = slice(c * 512, (c + 1) * 512)
                    nc.tensor.matmul(ps[:], lhsT=wslice(0, qoff, P),
                                     rhs=xqT[0][:, sl], start=True, stop=False)
                    nc.tensor.matmul(ps[:], lhsT=wslice(1, qoff, P),
                                     rhs=xqT[1][:, sl], start=False, stop=True)
                    if c % 2 == 0:
                        nc.scalar.add(QT[dout][:, sl], ps[:],
                                      bcol_sb[:, dout:dout + 1])
                    else:
                        nc.vector.tensor_scalar_add(
                            QT[dout][:, sl], ps[:],
                            bcol_sb[:, dout:dout + 1])

            # ---- attention ----
            for qb in range(nqb):
                qsl = slice(qb * 512, (qb + 1) * 512)
                av = [avpool.tile([P, 257], F32, tag="av", name=f"av{qb}_{i}")
                      for i in range(4)]

                def emit_av(et, sb):
                    for tt in range(4):
                        nc.tensor.matmul(
                            av[tt][:],
                            lhsT=et[:, tt * P:(tt + 1) * P],
                            rhs=V[:, sb * 257:(sb + 1) * 257],
                            start=(sb == 0), stop=(sb == nsb - 1))

                pending = None
                for sb in range(nsb):
                    sc = scpool.tile([P, 512], F32, tag="sc", name="sc")
                    ssl = slice(sb * P, (sb + 1) * P)
                    nc.tensor.matmul(sc[:], lhsT=KT[0][:, ssl],
                                     rhs=QT[0][:, qsl], start=True, stop=False)
                    nc.tensor.matmul(sc[:], lhsT=KT[1][:, ssl],
                                     rhs=QT[1][:, qsl], start=False, stop=True)
                    et = epool.tile([P, 512], DT, tag="exp", name="et")
                    nc.scalar.activation(et[:], sc[:], Exp, scale=SCALE)
                    if pending is not None:
                        emit_av(*pending)
                    pending = (et, sb)
                emit_av(*pending)

                # ---- normalize + store ----
                for tt in range(4):
                    rec = fpool.tile([P, 1], F32, tag="rec", name="rec")
                    nc.vector.reciprocal(rec[:], av[tt][:, 256:257])
                    ot = fpool.tile([P, D], F32, tag="ot", name="ot")
                    nc.vector.tensor_scalar_mul(ot[:], av[tt][:, 0:256], rec[:])
                    r0 = (qb * 4 + tt) * P
                    nc.sync.dma_start(out=out_e[r0:r0 + P, :], in_=ot[:])
    nc.finalize()
    return nc


def _get_nc(tk: int):
    if tk not in _nc_cache:
        _nc_cache[tk] = _build(tk)
    return _nc_cache[tk]


def _prep_in_maps(x, W, b, attention_mask):
    """Host-side layout prep + sharding. Returns (in_maps, tk)."""
    # de-interleave: W row 3d+n is (q,k,v)[n] output-dim d
    Wq, Wk, Wv = W[0::3], W[1::3], W[2::3]          # each [D, D]
    bq, bk, bv = b[0::3], b[1::3], b[2::3]
    wcat = np.concatenate(
        [Wq.T, Wk.T, Wv.T, np.zeros((D, 1), np.float32)], axis=1)
    biasones = np.concatenate(
        [np.zeros(512, np.float32), bv, [1.0],
         np.ones(1024, np.float32)])[None, :]
    wcat = np.ascontiguousarray(wcat).astype(NPDT)
    biasones = np.ascontiguousarray(biasones).astype(NPDT)
    biascols = np.ascontiguousarray(
        np.stack([bq[:P], bq[P:], bk[:P], bk[P:]], axis=1).astype(np.float32))

    if COMPACT:
        keep = [np.nonzero(attention_mask[bi] != 0)[0] for bi in range(B)]
        max_keep = max(len(k) for k in keep)
        tk = max(256, ((max_keep + 127) // 128) * 128)
    else:
        keep = None
        tk = T

    xkT_b, mask_b = [], []
    for bi in range(B):
        xb = x[bi]                                   # [T, D] f32
        if COMPACT:
            k = keep[bi]
            xs = np.zeros((tk, D), np.float32)
            xs[:len(k)] = xb[k]
            mf = np.zeros(tk, np.float32)
            mf[:len(k)] = 1.0
        else:
            xs = xb
            mf = attention_mask[bi].astype(np.float32)
        xkT_b.append(np.ascontiguousarray(xs.T.astype(NPDT)))
        mask_b.append(np.ascontiguousarray(
            mf.reshape(tk // P, P).T.astype(np.float32)))  # [P, nsb]

    in_maps = []
    for c in range(N_CORES):
        bi, half = c // 2, c % 2
        xqT = np.ascontiguousarray(
            x[bi][half * TQ:(half + 1) * TQ].T.astype(NPDT))
        in_maps.append({
            "xkT": xkT_b[bi],
            "xqT": xqT,
            "wcat": wcat,
            "biasones": biasones,
            "biascols": biascols,
            "maskf": mask_b[bi],
        })
    return in_maps, tk


def _gather(results) -> np.ndarray:
    out = np.empty((B, T, D), np.float32)
    for c in range(N_CORES):
        bi, half = c // 2, c % 2
        out[bi, half * TQ:(half + 1) * TQ] = results[c]["out"]
    return out


def kernel(x, W, b, attention_mask) -> np.ndarray:
    x = np.asarray(x, np.float32)
    W = np.asarray(W, np.float32)
    b = np.asarray(b, np.float32)
    attention_mask = np.asarray(attention_mask)
    in_maps, tk = _prep_in_maps(x, W, b, attention_mask)
    nc = _get_nc(tk)
    res = run_bass_kernel_spmd(nc, in_maps, list(range(N_CORES)))
    return _gather(res.results)
